# revision 3
# baseline (speedup 1.0000x reference)
"""Batched dense attention (B=16, S=2048, D=128) for 8 Trainium2 NeuronCores.

Strategy:
  - Pure data parallel over batch: 2 examples per core, SPMD NEFF on cores 0-7.
  - Per example, flash-style attention computed in "S^T layout":
      S^T[k, q] = (K^T chunk)^T-contracted-over-d with Q^T   (PE, fp16)
      E = exp(S^T / sqrt(D))                                 (ACT, PSUM->SBUF fp16)
      U^T[d, q] += V_chunk^T-contracted-over-k with E        (PE, fp16, PSUM fp32 accum)
      r[q] = sum_k E[k, q] via DVE chunk-accumulate + ones-matmul broadcast
      O^T = U^T * (1/r)                                      (DVE)
      O = transpose(O^T) per 128-block (PE) -> fp32 -> DRAM
  - Q^T / K^T obtained via fp16 cast (SWDGE cast-DMA to DRAM scratch) followed
    by hardware DMA-transpose (xbar) loads. V is cast-DMA'd straight into SBUF
    in its natural [k, d] chunk layout (it is the matmul stationary operand).

Numerics (validated against fp64 on host): global rel L2 err ~4.4e-4.
exp() without max-subtraction is safe: logits ~ N(0,1), observed |max| < 8,
theoretical bound 11.31; fp16 E max 65504 = exp(11.09).
"""

import numpy as np

B, S, D = 16, 2048, 128
NCORES = 8
BPC = B // NCORES  # batches per core
INV_SCALE = float(np.sqrt(D) + np.sqrt(D - D))  # sqrt(Dq) + sqrt(Dk-Dq) = 11.3137
SCALE = 1.0 / INV_SCALE
QB = 1024            # q-block (half of S): PSUM budget driven
NQB = S // QB        # 2
KC = 128             # k contraction chunk
NKC = S // KC        # 16
MMN = 512            # moving free dim per matmul (one PSUM bank)

_STATE = {}


def _build_nc():
    import concourse.bacc as bacc
    import concourse.tile as tile
    from concourse import mybir
    from concourse.masks import make_identity

    fp32 = mybir.dt.float32
    fp16 = mybir.dt.float16
    AF = mybir.ActivationFunctionType

    nc = bacc.Bacc(
        "TRN2",
        target_bir_lowering=False,
        debug=False,
        enable_asserts=False,
        num_devices=NCORES,
    )
    q = nc.dram_tensor("q", [BPC, S, D], fp32, kind="ExternalInput").ap()
    k = nc.dram_tensor("k", [BPC, S, D], fp32, kind="ExternalInput").ap()
    v = nc.dram_tensor("v", [BPC, S, D], fp32, kind="ExternalInput").ap()
    o = nc.dram_tensor("o", [BPC, S, D], fp32, kind="ExternalOutput").ap()

    with tile.TileContext(nc) as tc:
        with (
            tc.tile_pool(name="consts", bufs=1) as consts,
            tc.tile_pool(name="qkt", bufs=2) as qkt_pool,
            tc.tile_pool(name="vhp", bufs=2) as vh_pool,
            tc.tile_pool(name="ep", bufs=4) as e_pool,
            tc.tile_pool(name="accp", bufs=2) as acc_pool,
            tc.tile_pool(name="rp", bufs=2) as r_pool,
            tc.tile_pool(name="otp", bufs=2) as ot_pool,
            tc.tile_pool(name="obp", bufs=4) as ob_pool,
            tc.tile_pool(name="dram", bufs=2, space="DRAM") as dram_pool,
            tc.tile_pool(name="ps", bufs=2, space="PSUM") as ps_pool,
            tc.tile_pool(name="pu", bufs=1, space="PSUM") as pu_pool,
            tc.tile_pool(name="po", bufs=2, space="PSUM") as po_pool,
        ):
            ident = consts.tile([128, 128], fp16)
            make_identity(nc, ident)
            ones = consts.tile([128, 128], fp16)
            nc.vector.memset(ones, 1.0)

            for b in range(BPC):
                # fp16 staging of Q/K in DRAM, then xbar-transposed loads.
                qs = dram_pool.tile([S, D], fp16, tag="qs")
                ks = dram_pool.tile([S, D], fp16, tag="ks")
                nc.gpsimd.dma_start(out=qs, in_=q[b])
                nc.gpsimd.dma_start(out=ks, in_=k[b])
                qt = qkt_pool.tile([128, S], fp16, tag="qt")
                kt = qkt_pool.tile([128, S], fp16, tag="kt")
                nc.sync.dma_start_transpose(qt, qs[:])
                nc.sync.dma_start_transpose(kt, ks[:])
                # V in natural chunk layout [p=k_in_chunk, chunk, d], fp16.
                vh = vh_pool.tile([128, NKC, KC], fp16, tag="vh")
                nc.gpsimd.dma_start(
                    out=vh, in_=v[b].rearrange("(t p) d -> p t d", p=128)
                )

                for h in range(NQB):
                    u = pu_pool.tile([128, QB], fp32, tag="u")
                    acc = acc_pool.tile([128, QB], fp16, tag="acc")
                    for c in range(NKC):
                        st = ps_pool.tile([128, QB], fp32, tag="st")
                        for j in range(QB // MMN):
                            nc.tensor.matmul(
                                st[:, j * MMN : (j + 1) * MMN],
                                lhsT=kt[:, c * KC : (c + 1) * KC],
                                rhs=qt[:, h * QB + j * MMN : h * QB + (j + 1) * MMN],
                                start=True,
                                stop=True,
                            )
                        e = e_pool.tile([128, QB], fp16, tag="e")
                        nc.scalar.activation(out=e, in_=st[:], func=AF.Exp, scale=SCALE)
                        for j in range(QB // MMN):
                            nc.tensor.matmul(
                                u[:, j * MMN : (j + 1) * MMN],
                                lhsT=vh[:, c, :],
                                rhs=e[:, j * MMN : (j + 1) * MMN],
                                start=(c == 0),
                                stop=(c == NKC - 1),
                                skip_group_check=True,
                            )
                        if c == 0:
                            nc.vector.tensor_copy(out=acc[:], in_=e[:])
                        else:
                            nc.vector.tensor_add(acc[:], acc[:], e[:])
                    # r[q] broadcast across partitions via ones-matmul, then 1/r.
                    rbc = ps_pool.tile([128, QB], fp32, tag="st")
                    for j in range(QB // MMN):
                        nc.tensor.matmul(
                            rbc[:, j * MMN : (j + 1) * MMN],
                            lhsT=ones[:],
                            rhs=acc[:, j * MMN : (j + 1) * MMN],
                            start=True,
                            stop=True,
                        )
                    rrec = r_pool.tile([128, QB], fp32, tag="rrec")
                    nc.vector.reciprocal(out=rrec[:], in_=rbc[:])
                    ot = ot_pool.tile([128, QB], fp16, tag="ot")
                    nc.vector.tensor_mul(ot[:], u[:], rrec[:])
                    for t in range(QB // 128):
                        pot = po_pool.tile([128, 128], fp16, tag="pot")
                        nc.tensor.transpose(pot[:], ot[:, t * 128 : (t + 1) * 128], ident[:])
                        ob = ob_pool.tile([128, 128], fp32, tag="ob")
                        nc.vector.tensor_copy(out=ob[:], in_=pot[:])
                        row0 = (h * (QB // 128) + t) * 128
                        nc.sync.dma_start(out=o[b, row0 : row0 + 128, :], in_=ob[:])

    nc.compile()
    return nc


def _get_nc():
    if "nc" not in _STATE:
        _STATE["nc"] = _build_nc()
    return _STATE["nc"]


def kernel(query, key, value):
    from concourse import bass_utils

    nc = _get_nc()
    query = np.ascontiguousarray(query, dtype=np.float32)
    key = np.ascontiguousarray(key, dtype=np.float32)
    value = np.ascontiguousarray(value, dtype=np.float32)
    in_maps = [
        {
            "q": query[i * BPC : (i + 1) * BPC],
            "k": key[i * BPC : (i + 1) * BPC],
            "v": value[i * BPC : (i + 1) * BPC],
        }
        for i in range(NCORES)
    ]
    res = bass_utils.run_bass_kernel_spmd(
        nc,
        in_maps,
        core_ids=list(range(NCORES)),
        trace=_STATE.get("trace", False),
    )
    _STATE["last_results"] = res
    return np.concatenate([res.results[i]["o"] for i in range(NCORES)], axis=0)


# revision 6
# speedup vs baseline: 1.5213x; 1.5213x over previous
"""Batched dense attention (B=16, S=2048, D=128) for 8 Trainium2 NeuronCores.

Strategy:
  - Pure data parallel over batch: 2 examples per core, SPMD NEFF on cores 0-7.
  - Per example, attention computed in "S^T layout" (k on partitions, q on free):
      S^T[k, q] = matmul(lhsT=K^T chunk, rhs=Q^T)            (PE, bf16)
      E = exp(S^T / sqrt(D))                                 (ACT, PSUM->SBUF bf16)
      U^T[d, q] += matmul(lhsT=V chunk, rhs=E)               (PE, bf16, fp32 PSUM accum)
      acc[kk, q] += E chunk                                  (DVE, fp16)
      r-broadcast = matmul(lhsT=ones, rhs=acc)               (PE)
      O^T = U^T * reciprocal_approx_fast(r)                  (DVE, fp16 out)
      O^T -> DRAM -> xbar DMA-transpose -> [q, d] tiles -> SWDGE cast-DMA fp32 out
  - Q^T / K^T: HWDGE load fp32 -> DVE cast bf16 -> HWDGE store to DRAM scratch
    -> hardware DMA-transpose (xbar) load. Pipelined at half-tensor granularity.
  - V: HWDGE load fp32 -> DVE cast bf16 in natural [k-chunk, d] layout (stationary
    matmul operand).

Numerics: logits bf16 (~0.4% elem rounding on Q/K), E bf16, acc fp16, output fp16.
exp() without max-subtraction is safe: logits ~ N(0,1) (scale 1/sqrt(128)),
theoretical |logit| <= 11.31, observed < 8; fp16/bf16 exp range is fine.
"""

import numpy as np

B, S, D = 16, 2048, 128
NCORES = 8
BPC = B // NCORES  # batches per core
INV_SCALE = float(np.sqrt(D) + np.sqrt(D - D))  # sqrt(Dq) + sqrt(Dk-Dq)
SCALE = 1.0 / INV_SCALE
QB = 1024            # q-block (half of S): PSUM budget driven
NQB = S // QB        # 2
KC = 128             # k contraction chunk
NKC = S // KC        # 16
MMN = 512            # moving free dim per matmul (one PSUM bank)
NT = QB // 128       # 8 output tiles per q-block

_STATE = {}


def _build_nc():
    import concourse.bacc as bacc
    import concourse.tile as tile
    from concourse import mybir

    fp32 = mybir.dt.float32
    bf16 = mybir.dt.bfloat16
    fp16 = mybir.dt.float16
    AF = mybir.ActivationFunctionType

    nc = bacc.Bacc(
        "TRN2",
        target_bir_lowering=False,
        debug=False,
        enable_asserts=False,
        num_devices=NCORES,
    )
    q = nc.dram_tensor("q", [BPC, S, D], fp32, kind="ExternalInput").ap()
    k = nc.dram_tensor("k", [BPC, S, D], fp32, kind="ExternalInput").ap()
    v = nc.dram_tensor("v", [BPC, S, D], fp32, kind="ExternalInput").ap()
    o = nc.dram_tensor("o", [BPC, S, D], fp32, kind="ExternalOutput").ap()

    with tile.TileContext(nc) as tc:
        with (
            tc.tile_pool(name="consts", bufs=1) as consts,
            tc.tile_pool(name="stage", bufs=3) as stage_pool,     # fp32 input staging
            tc.tile_pool(name="h16", bufs=3) as h16_pool,         # bf16 pre-transpose
            tc.tile_pool(name="qkt", bufs=2) as qkt_pool,         # Q^T / K^T bf16
            tc.tile_pool(name="vhp", bufs=2) as vh_pool,
            tc.tile_pool(name="ep", bufs=4) as e_pool,
            tc.tile_pool(name="accp", bufs=2) as acc_pool,
            tc.tile_pool(name="rp", bufs=2) as r_pool,
            tc.tile_pool(name="otp", bufs=2) as ot_pool,          # O^T fp16
            tc.tile_pool(name="obp", bufs=2) as ob_pool,          # [q, d] tiles fp16
            tc.tile_pool(name="dram", bufs=2, space="DRAM") as dram_pool,
            tc.tile_pool(name="ps", bufs=2, space="PSUM") as ps_pool,
            tc.tile_pool(name="pu", bufs=1, space="PSUM") as pu_pool,
            tc.tile_pool(name="pr", bufs=1, space="PSUM") as pr_pool,
        ):
            ones = consts.tile([128, 128], fp16)
            nc.vector.memset(ones, 1.0)

            for b in range(BPC):
                # ---- input pipeline (half-tensor granularity) ----
                # staging loads in natural [p, t, d] chunk layout
                qst = stage_pool.tile([128, NKC, KC], fp32, tag="qst")
                kst = stage_pool.tile([128, NKC, KC], fp32, tag="kst")
                vst = stage_pool.tile([128, NKC, KC], fp32, tag="vst")
                qh16 = h16_pool.tile([128, NKC, KC], bf16, tag="qh16")
                kh16 = h16_pool.tile([128, NKC, KC], bf16, tag="kh16")
                vh = vh_pool.tile([128, NKC, KC], bf16, tag="vh")
                qscr = dram_pool.tile([S, D], bf16, tag="qscr")
                kscr = dram_pool.tile([S, D], bf16, tag="kscr")
                qt = qkt_pool.tile([128, S], bf16, tag="qt")
                kt = qkt_pool.tile([128, S], bf16, tag="kt")

                HT = NKC // 2  # 8 chunks per half
                for hh in range(2):
                    ts_ = slice(hh * HT, (hh + 1) * HT)
                    rs = slice(hh * (S // 2), (hh + 1) * (S // 2))
                    for name, st, h16t, src in (
                        ("k", kst, kh16, k),
                        ("q", qst, qh16, q),
                        ("v", vst, vh, v),
                    ):
                        nc.sync.dma_start(
                            out=st[:, ts_, :],
                            in_=src[b].rearrange("(t p) d -> p t d", p=128)[:, ts_, :],
                        )
                        nc.vector.tensor_copy(out=h16t[:, ts_, :], in_=st[:, ts_, :])
                    for scr, h16t in ((kscr, kh16), (qscr, qh16)):
                        nc.sync.dma_start(
                            out=scr.rearrange("(t p) d -> p t d", p=128)[:, ts_, :],
                            in_=h16t[:, ts_, :],
                        )
                    nc.sync.dma_start_transpose(kt[:, rs], kscr[rs, :])
                    nc.sync.dma_start_transpose(qt[:, rs], qscr[rs, :])

                # ---- main attention loop ----
                for h in range(NQB):
                    qs_ = slice(h * QB, (h + 1) * QB)
                    u = pu_pool.tile([128, QB], fp32, tag="u")
                    acc = acc_pool.tile([128, QB], fp16, tag="acc")
                    for c in range(NKC):
                        st = ps_pool.tile([128, QB], fp32, tag="st")
                        for j in range(QB // MMN):
                            nc.tensor.matmul(
                                st[:, j * MMN : (j + 1) * MMN],
                                lhsT=kt[:, c * KC : (c + 1) * KC],
                                rhs=qt[:, h * QB + j * MMN : h * QB + (j + 1) * MMN],
                                start=True,
                                stop=True,
                            )
                        e = e_pool.tile([128, QB], bf16, tag="e")
                        nc.scalar.activation(out=e, in_=st[:], func=AF.Exp, scale=SCALE)
                        for j in range(QB // MMN):
                            nc.tensor.matmul(
                                u[:, j * MMN : (j + 1) * MMN],
                                lhsT=vh[:, c, :],
                                rhs=e[:, j * MMN : (j + 1) * MMN],
                                start=(c == 0),
                                stop=(c == NKC - 1),
                                skip_group_check=True,
                            )
                        if c == 0:
                            nc.vector.tensor_copy(out=acc[:], in_=e[:])
                        else:
                            nc.vector.tensor_add(acc[:], acc[:], e[:])
                    # r[q] broadcast across partitions via ones-matmul, then 1/r.
                    rbc = pr_pool.tile([128, QB], fp32, tag="rbc")
                    for j in range(QB // MMN):
                        nc.tensor.matmul(
                            rbc[:, j * MMN : (j + 1) * MMN],
                            lhsT=ones[:],
                            rhs=acc[:, j * MMN : (j + 1) * MMN],
                            start=True,
                            stop=True,
                        )
                    rrec = r_pool.tile([128, QB], fp32, tag="rrec")
                    nc.vector.reciprocal_approx_fast(out=rrec[:], in_=rbc[:])
                    ot = ot_pool.tile([128, QB], fp16, tag="ot")
                    nc.vector.tensor_mul(ot[:], u[:], rrec[:])
                    # O^T -> DRAM -> xbar transpose -> [q, d] tiles -> fp32 out
                    oscr = dram_pool.tile([128, QB], fp16, tag="oscr")
                    nc.sync.dma_start(out=oscr[:], in_=ot[:])
                    ob = ob_pool.tile([128, NT, 128], fp16, tag="ob")
                    nc.sync.dma_start_transpose(ob[:], oscr[:])
                    # ob's extra dim extends the partition dim: ob[p, t, :] holds
                    # transpose-row t*128+p, i.e. O[q = t*128 + p, :].
                    nc.gpsimd.dma_start(
                        out=o[b, qs_, :].rearrange("(t p) d -> p t d", p=128),
                        in_=ob[:],
                    )

    nc.compile()
    return nc


def _get_nc():
    if "nc" not in _STATE:
        _STATE["nc"] = _build_nc()
    return _STATE["nc"]


def kernel(query, key, value):
    from concourse import bass_utils

    nc = _get_nc()
    query = np.ascontiguousarray(query, dtype=np.float32)
    key = np.ascontiguousarray(key, dtype=np.float32)
    value = np.ascontiguousarray(value, dtype=np.float32)
    in_maps = [
        {
            "q": query[i * BPC : (i + 1) * BPC],
            "k": key[i * BPC : (i + 1) * BPC],
            "v": value[i * BPC : (i + 1) * BPC],
        }
        for i in range(NCORES)
    ]
    res = bass_utils.run_bass_kernel_spmd(
        nc,
        in_maps,
        core_ids=list(range(NCORES)),
        trace=_STATE.get("trace", False),
    )
    _STATE["last_results"] = res
    return np.concatenate([res.results[i]["o"] for i in range(NCORES)], axis=0)
